# revision 1
# baseline (speedup 1.0000x reference)
"""Trainium2 Bass kernel for nn_ATTN_86543591014439 (dense transformer block).

Reference computation (B=32, S=256, OBS=64, D=1024, H=16 heads, HD=64, A=18):
  h   = x @ W_obs.T + b_obs + pos_emb            [B,S,D]
  qkv = h @ in_w.T + in_b; causal 16-head self-attention
  o   = attn_out @ out_w.T + out_b;  h = h + relu(o)
  f   = relu(h @ w1.T + b1) @ w2.T + b2;  h = h + relu(f)
  out = h @ wa.T + ba                            [B,S,A]

Strategy: data-parallel over batch. 8 cores x 4 sequences (T=1024 token rows
per core), weights replicated, no collectives. All activations stay in SBUF in
feature-major layout ("xT" = [feat, tok]); matmuls run in float32r (fp32 data
rounded to ~tf32 by the producing instruction, 1 PE cycle/row when the moving
dim is >= 256 - ~4x faster than plain fp32, ~1e-4 relative error).

Attention per (seq b, head h): scoresT[k,q] = k-slice.T @ q-slice (K=64, both
feature-major from qkT); expT = Exp(scoresT/8) masked by causal maskT on
gpsimd; token-major V carries an appended ones column so one accumulating
matmul yields both u = V.T @ expT and the softmax denominators (psum row 64).
Denominators of 2 consecutive pairs are gathered to SBUF partitions {0,1}
(single-row ACT/DVE copies move across partitions), one DVE reciprocal serves
both, a K=2 ones matmul broadcasts both reciprocal rows to a [128,256] tile,
and two DVE muls write the normalized oT slices. No PE transposes anywhere.
"""

import numpy as np

import concourse.tile as tile
from concourse import bacc, mybir
from concourse.bass_utils import run_bass_kernel_spmd

F32 = mybir.dt.float32
F32R = mybir.dt.float32r

B, S, OBS, D, H, A = 32, 256, 64, 1024, 16, 18
HD = D // H
NCORES = 8
BC = B // NCORES  # sequences per core
T = BC * S  # token rows per core (1024)
KC = D // 128  # 128-chunks over D
AF = mybir.ActivationFunctionType

_cache = {}


def _build_nc():
    nc = bacc.Bacc()

    def inp(name, shape, dtype=F32R):
        return nc.declare_dram_parameter(name, list(shape), dtype, isOutput=False).ap()

    xT_e = inp("xT", [OBS, T])
    wobs_e = inp("wobsT", [OBS, D])
    pos_e = inp("posT4b", [D, S], F32)
    wqk_e = inp("in_wT_qk", [D, 2 * D])
    inbqk_e = inp("inb_qk", [128, 16], F32)
    wv_e = inp("in_wT_v", [D, D])
    wo_e = inp("out_wT", [D, D])
    outb_e = inp("outb", [128, KC], F32)
    w1_e = inp("w1T", [D, 4 * D])
    b1_e = inp("b1", [128, 32], F32)
    w2_e = inp("w2T", [4 * D, D])
    b2_e = inp("b2", [128, KC], F32)
    wa_e = inp("waT", [D, A])
    baB_e = inp("baB", [128, A], F32)
    mask_e = inp("maskT", [128, 2, S])
    ones_v_e = inp("ones_v", [128, H])
    onesb_e = inp("onesb", [128, 64])
    out_e = nc.declare_dram_parameter("out", [T, A], F32, isOutput=True).ap()

    with tile.TileContext(nc) as tc:
        with (
            tc.tile_pool(name="cpool", bufs=1) as cpool,
            tc.tile_pool(name="htp", bufs=1) as htp,
            tc.tile_pool(name="rpool", bufs=3) as rpool,
        ):
            # const tiles allocated up front; their DMAs are emitted after
            # phase E so E's xT/wobs/pos transfers lead the DMA queue
            maskT = cpool.tile([128, 2, S], F32R)
            onesB = cpool.tile([128, 64], F32R)
            inb = cpool.tile([128, 16], F32)
            outb = cpool.tile([128, KC], F32)
            b1 = cpool.tile([128, 32], F32)
            b2 = cpool.tile([128, KC], F32)
            baB = cpool.tile([128, A], F32)
            wa = cpool.tile([128, KC, A], F32R)

            ht = [
                htp.tile([128, T], F32R, tag=f"ht{m}", name=f"ht{m}")
                for m in range(KC)
            ]

            with (
                tc.tile_pool(name="qkp", bufs=1) as qkp,
                tc.tile_pool(name="vtp", bufs=1) as vtp,
            ):
                qk = [
                    qkp.tile([128, T], F32R, tag=f"qk{m}", name=f"qk{m}")
                    for m in range(16)
                ]
                vt = [
                    vtp.tile([128, H, HD + 1], F32R, tag=f"vt{m}", name=f"vt{m}")
                    for m in range(8)
                ]

                with (
                    tc.tile_pool(name="psg1", bufs=2, space="PSUM") as psg1,
                    tc.tile_pool(name="wvp", bufs=2) as wvp,
                ):
                    wv_r = wv_e.rearrange("(kc p) n -> p kc n", p=128)
                    wv = [
                        wvp.tile([128, KC, 512], F32R, tag="wv", name=f"wv{vc}")
                        for vc in range(2)
                    ]
                    # ---- E: hT = W_obs @ xT + (pos + b_obs) ----
                    with (
                        nc.named_scope("E"),
                        tc.tile_pool(name="exw", bufs=1) as exw,
                        tc.tile_pool(name="ppos", bufs=8) as ppos,
                    ):
                        xT = exw.tile([OBS, T], F32R)
                        nc.sync.dma_start(out=xT, in_=xT_e)
                        wobs = exw.tile([OBS, D], F32R)
                        nc.sync.dma_start(out=wobs, in_=wobs_e)
                        poss = []
                        for m in range(KC):
                            pos = ppos.tile(
                                [128, S], F32, tag="pos", name=f"pos{m}"
                            )
                            nc.sync.dma_start(
                                out=pos, in_=pos_e[m * 128 : (m + 1) * 128, 0:S]
                            )
                            poss.append(pos)
                        # prefetch V's first weight block under phase E
                        nc.sync.dma_start(out=wv[0], in_=wv_r[:, :, 0:512])
                        for m in range(KC):
                            pos = poss[m]
                            for tcol in range(T // 512):
                                sl = slice(tcol * 512, (tcol + 1) * 512)
                                ps = psg1.tile([128, 512], F32, tag="ps")
                                nc.tensor.matmul(
                                    ps,
                                    wobs[:, m * 128 : (m + 1) * 128],
                                    xT[:, sl],
                                    start=True,
                                    stop=True,
                                )
                                for q in range(2):
                                    nc.vector.tensor_add(
                                        ht[m][:, 2 * tcol * S + q * S : 2 * tcol * S + (q + 1) * S],
                                        ps[:, q * S : (q + 1) * S],
                                        pos,
                                    )

                    nc.sync.dma_start(out=inb, in_=inbqk_e)
                    nc.sync.dma_start(out=outb, in_=outb_e)
                    nc.sync.dma_start(out=b1, in_=b1_e)
                    nc.sync.dma_start(out=b2, in_=b2_e)
                    nc.sync.dma_start(out=baB, in_=baB_e)
                    nc.sync.dma_start(
                        out=wa, in_=wa_e.rearrange("(kc p) a -> p kc a", p=128)
                    )

                    # ---- V: token-major v = h @ Wv.T, with ones column ----
                    with nc.named_scope("V"):
                        for mt in range(8):
                            nc.sync.dma_start(
                                out=vt[mt][:, :, HD : HD + 1],
                                in_=ones_v_e.unsqueeze(2),
                            )
                        nc.sync.dma_start(out=wv[1], in_=wv_r[:, :, 512:1024])
                        for vc in range(2):
                            for mt in range(8):
                                ps = psg1.tile([128, 512], F32, tag="ps")
                                for k in range(KC):
                                    nc.tensor.matmul(
                                        ps,
                                        ht[k][:, mt * 128 : (mt + 1) * 128],
                                        wv[vc][:, k, :],
                                        start=(k == 0),
                                        stop=(k == KC - 1),
                                    )
                                nc.scalar.activation(
                                    out=vt[mt][:, 8 * vc : 8 * vc + 8, 0:HD],
                                    in_=ps.rearrange("p (h d) -> p h d", d=HD),
                                    func=AF.Copy,
                                )

                with tc.tile_pool(name="otp", bufs=1) as otp:
                    ot = [
                        otp.tile([128, T], F32R, tag=f"ot{m}", name=f"ot{m}")
                        for m in range(KC)
                    ]

                    # ---- QA: Q blocks interleaved with attention pairs.
                    # Pairs for head-pair hh immediately follow Q(hh)/Q(8+hh)
                    # (earlier program order = higher priority), so later Q
                    # blocks act as PE gap-fillers under the attention chains.
                    with (
                        nc.named_scope("QA"),
                        tc.tile_pool(name="wqkp", bufs=3) as wqkp,
                        tc.tile_pool(name="apool", bufs=1) as apool,
                        tc.tile_pool(name="aps", bufs=4, space="PSUM") as aps,
                        tc.tile_pool(name="apu", bufs=4, space="PSUM") as apu,
                    ):
                        nc.sync.dma_start(out=maskT, in_=mask_e)
                        nc.sync.dma_start(out=onesB, in_=onesb_e)
                        wqk_r = wqk_e.rearrange("(kc p) m -> p kc m", p=128)

                        def emit_q_block(m):
                            wqk = wqkp.tile(
                                [128, KC, 128], F32R, tag="wqk", name=f"wqk{m}"
                            )
                            nc.sync.dma_start(
                                out=wqk, in_=wqk_r[:, :, m * 128 : (m + 1) * 128]
                            )
                            for tcol in range(T // 512):
                                sl = slice(tcol * 512, (tcol + 1) * 512)
                                ps = aps.tile(
                                    [128, 512], F32, tag="ps_s", name=f"psq{m}_{tcol}"
                                )
                                for k in range(KC):
                                    nc.tensor.matmul(
                                        ps,
                                        wqk[:, k, :],
                                        ht[k][:, sl],
                                        start=(k == 0),
                                        stop=(k == KC - 1),
                                    )
                                nc.scalar.activation(
                                    out=qk[m][:, sl],
                                    in_=ps,
                                    func=AF.Identity,
                                    bias=inb[:, m : m + 1],
                                    scale=1.0,
                                )

                        for hh in range(8):
                            emit_q_block(hh)
                            emit_q_block(8 + hh)
                            psu_g = [None] * 2
                            scoll = None
                            qt = qk[hh]
                            kt = qk[8 + hh]
                            for pi in range(2 * BC):
                                b, g = pi // 2, pi % 2
                                idx = pi % 2
                                h = 2 * hh + g
                                r0 = g * 64
                                ps_s = aps.tile(
                                    [128, 2, S],
                                    F32,
                                    tag="ps_s",
                                    name=f"ps_s{b}_{h}",
                                )
                                for kc in range(2):
                                    c0 = b * S + kc * 128
                                    nc.tensor.matmul(
                                        ps_s[:, kc, :],
                                        kt[r0 : r0 + 64, c0 : c0 + 128],
                                        qt[r0 : r0 + 64, b * S : (b + 1) * S],
                                        start=True,
                                        stop=True,
                                    )
                                expT = apool.tile(
                                    [128, 2, S],
                                    F32R,
                                    tag="expT", bufs=5,
                                    name=f"expT{b}_{h}",
                                )
                                nc.scalar.activation(
                                    out=expT.rearrange("p a b -> p (a b)"),
                                    in_=ps_s.rearrange("p a b -> p (a b)"),
                                    func=AF.Exp,
                                    scale=0.125,
                                )
                                # chunk0: only q<128 needs the causal mask;
                                # chunk1 mask also zeroes its dead q<128
                                nc.gpsimd.tensor_mul(
                                    expT[:, 0, 0:128],
                                    expT[:, 0, 0:128],
                                    maskT[:, 0, 0:128],
                                )
                                nc.gpsimd.tensor_mul(
                                    expT[:, 1, :], expT[:, 1, :], maskT[:, 1, :]
                                )
                                ps_u = apu.tile(
                                    [65, S], F32, tag="ps_ub", name=f"ps_u{b}_{h}"
                                )
                                for kc in range(2):
                                    nc.tensor.matmul(
                                        ps_u,
                                        vt[2 * b + kc][:, h, :],
                                        expT[:, kc, :],
                                        start=(kc == 0),
                                        stop=(kc == 1),
                                    )
                                psu_g[idx] = ps_u
                                if idx == 0:
                                    scoll = apool.tile(
                                        [128, S],
                                        F32R,
                                        tag="scoll", bufs=2,
                                        name=f"scoll{b}_{hh}",
                                    )
                                # gather sums rows (psum partition 64) onto
                                # partitions {0,32,64,96}: one reciprocal
                                # then serves 4 pairs
                                if idx % 2 == 0:
                                    nc.scalar.activation(
                                        out=scoll[64 * idx : 64 * idx + 1, :],
                                        in_=ps_u[64:65, :],
                                        func=AF.Copy,
                                    )
                                else:
                                    nc.vector.tensor_copy(
                                        out=scoll[64 * idx : 64 * idx + 1, :],
                                        in_=ps_u[64:65, :],
                                    )
                                if idx < 1:
                                    continue
                                rcoll = apool.tile(
                                    [128, S], F32R, tag="rcoll", bufs=2,
                                    name=f"rc{b}_{hh}",
                                )
                                with nc.allow_low_precision(
                                    reason="softmax recip"
                                ):
                                    nc.vector.reciprocal(out=rcoll, in_=scoll)
                                for gj in range(2):
                                    pj = pi - 1 + gj
                                    bj, g2 = pj // 2, pj % 2
                                    hj = 2 * hh + g2
                                    rj = g2 * 64
                                    ps_b = apu.tile(
                                        [64, S],
                                        F32,
                                        tag="ps_ub",
                                        name=f"ps_b{bj}_{hj}",
                                    )
                                    nc.tensor.matmul(
                                        ps_b,
                                        onesB[64 * gj : 64 * gj + 1, :],
                                        rcoll[64 * gj : 64 * gj + 1, :],
                                        start=True,
                                        stop=True,
                                        tile_position=(64 * gj, 0),
                                    )
                                    recipB = apool.tile(
                                        [64, S],
                                        F32,
                                        tag="recipB", bufs=3,
                                        name=f"rB{bj}_{hj}",
                                    )
                                    if gj % 2 == 0:
                                        nc.scalar.activation(
                                            out=recipB, in_=ps_b, func=AF.Copy
                                        )
                                    else:
                                        nc.vector.tensor_copy(
                                            out=recipB, in_=ps_b
                                        )
                                    nc.vector.tensor_mul(
                                        ot[hj // 2][
                                            rj : rj + 64, bj * S : (bj + 1) * S
                                        ],
                                        psu_g[gj][0:64, :],
                                        recipB,
                                    )

                    with (
                        nc.named_scope("P"),
                        tc.tile_pool(name="wop", bufs=3) as wop,
                        tc.tile_pool(name="psp", bufs=4, space="PSUM") as psp,
                    ):
                        wo_r = wo_e.rearrange("(kc p) m -> p kc m", p=128)
                        for m in range(KC):
                            wo = wop.tile([128, KC, 128], F32R, tag="wo")
                            nc.sync.dma_start(
                                out=wo, in_=wo_r[:, :, m * 128 : (m + 1) * 128]
                            )
                            for tcol in range(T // 512):
                                sl = slice(tcol * 512, (tcol + 1) * 512)
                                ps = psp.tile([128, 512], F32, tag="ps")
                                for k in range(KC):
                                    nc.tensor.matmul(
                                        ps,
                                        wo[:, k, :],
                                        ot[k][:, sl],
                                        start=(k == 0),
                                        stop=(k == KC - 1),
                                    )
                                rt = rpool.tile([128, 512], F32, tag="rt")
                                nc.scalar.activation(
                                    out=rt,
                                    in_=ps,
                                    func=AF.Relu,
                                    bias=outb[:, m : m + 1],
                                    scale=1.0,
                                )
                                nc.vector.tensor_add(
                                    ht[m][:, sl], ht[m][:, sl], rt
                                )

            # ---- F1/F2: FFN (otp/qkp/vtp scopes closed, SBUF freed) ----
            with tc.tile_pool(name="gtp", bufs=1) as gtp:
                gt = [
                    gtp.tile([128, T], F32R, tag=f"gt{m}", name=f"gt{m}")
                    for m in range(32)
                ]
                with tc.tile_pool(name="psg3", bufs=4, space="PSUM") as psg3:
                    with (
                        nc.named_scope("F1"),
                        tc.tile_pool(name="w1p", bufs=3) as w1p,
                    ):
                        w1_r = w1_e.rearrange("(kc p) m -> p kc m", p=128)
                        for m in range(32):
                            w1m = w1p.tile([128, KC, 128], F32R, tag="w1m")
                            nc.sync.dma_start(
                                out=w1m, in_=w1_r[:, :, m * 128 : (m + 1) * 128]
                            )
                            for tcol in range(T // 512):
                                sl = slice(tcol * 512, (tcol + 1) * 512)
                                ps = psg3.tile([128, 512], F32, tag="ps")
                                for k in range(KC):
                                    nc.tensor.matmul(
                                        ps,
                                        w1m[:, k, :],
                                        ht[k][:, sl],
                                        start=(k == 0),
                                        stop=(k == KC - 1),
                                    )
                                nc.scalar.activation(
                                    out=gt[m][:, sl],
                                    in_=ps,
                                    func=AF.Relu,
                                    bias=b1[:, m : m + 1],
                                    scale=1.0,
                                )

                    with (
                        nc.named_scope("F2"),
                        tc.tile_pool(name="w2p", bufs=2) as w2p,
                    ):
                        w2_r = w2_e.rearrange("(kc p) m -> p kc m", p=128)
                        for m in range(KC):
                            w2m = w2p.tile([128, 32, 128], F32R, tag="w2m")
                            for qtr in range(4):
                                nc.sync.dma_start(
                                    out=w2m[:, 8 * qtr : 8 * qtr + 8, :],
                                    in_=w2_r[
                                        :, 8 * qtr : 8 * qtr + 8,
                                        m * 128 : (m + 1) * 128,
                                    ],
                                )
                            for tcol in range(T // 512):
                                sl = slice(tcol * 512, (tcol + 1) * 512)
                                ps = psg3.tile([128, 512], F32, tag="ps")
                                for k in range(32):
                                    nc.tensor.matmul(
                                        ps,
                                        w2m[:, k, :],
                                        gt[k][:, sl],
                                        start=(k == 0),
                                        stop=(k == 31),
                                    )
                                rt = rpool.tile([128, 512], F32, tag="rt")
                                nc.scalar.activation(
                                    out=rt,
                                    in_=ps,
                                    func=AF.Relu,
                                    bias=b2[:, m : m + 1],
                                    scale=1.0,
                                )
                                nc.vector.tensor_add(ht[m][:, sl], ht[m][:, sl], rt)

                    # ---- H: out = h3 @ wa.T + ba (token-major) ----
                    with nc.named_scope("H"):
                        for mt in range(8):
                            ps = psg3.tile([128, 512], F32, tag="ps")
                            for k in range(KC):
                                nc.tensor.matmul(
                                    ps[:, 0:A],
                                    ht[k][:, mt * 128 : (mt + 1) * 128],
                                    wa[:, k, :],
                                    start=(k == 0),
                                    stop=(k == KC - 1),
                                )
                            os_ = rpool.tile([128, A], F32, tag="os")
                            nc.vector.tensor_add(os_, ps[:, 0:A], baB)
                            nc.sync.dma_start(
                                out=out_e[mt * 128 : (mt + 1) * 128, :], in_=os_
                            )

    nc.compile()
    return nc


def _host_prep(
    x, pos_emb, W_obs, b_obs, in_w, in_b, out_w, out_b, w1, b1, w2, b2, wa, ba
):
    f = np.float32

    shared = {
        "wobsT": np.ascontiguousarray(W_obs.T, dtype=f),
        "in_wT_qk": np.ascontiguousarray(in_w[: 2 * D].T, dtype=f),
        "inb_qk": np.ascontiguousarray(
            in_b[: 2 * D].reshape(16, 128).T, dtype=f
        ),
        "in_wT_v": np.ascontiguousarray(in_w[2 * D :].T, dtype=f),
        "out_wT": np.ascontiguousarray(out_w.T, dtype=f),
        "w1T": np.ascontiguousarray(w1.T, dtype=f),
        "b1": np.ascontiguousarray(b1.reshape(32, 128).T, dtype=f),
        "w2T": np.ascontiguousarray(w2.T, dtype=f),
        "b2": np.ascontiguousarray(b2.reshape(KC, 128).T, dtype=f),
        "waT": np.ascontiguousarray(wa.T, dtype=f),
        "baB": np.ascontiguousarray(np.broadcast_to(ba, (128, A)), dtype=f),
        "ones_v": np.ones((128, H), f),
        "onesb": np.ones((128, 64), f),
    }
    # v-bias commutes through attention (rows of attn sum to 1):
    # out_b_eff = out_b + bv @ out_w.T
    out_b_eff = out_b + in_b[2 * D :] @ out_w.T
    shared["outb"] = np.ascontiguousarray(
        np.asarray(out_b_eff, f).reshape(KC, 128).T, dtype=f
    )
    posT = np.asarray(pos_emb[0].T, f) + np.asarray(b_obs, f)[:, None]
    shared["posT4b"] = np.ascontiguousarray(posT, dtype=f)
    kidx = np.arange(2)[None, :, None] * 128 + np.arange(128)[:, None, None]
    qidx = np.arange(S)[None, None, :]
    shared["maskT"] = np.ascontiguousarray((kidx <= qidx).astype(f))

    in_maps = []
    for c in range(NCORES):
        m = dict(shared)
        xc = np.asarray(x[c * BC : (c + 1) * BC], f).reshape(T, OBS)
        m["xT"] = np.ascontiguousarray(xc.T)
        in_maps.append(m)
    return in_maps


def kernel(**inputs):
    if "nc" not in _cache:
        _cache["nc"] = _build_nc()
    nc = _cache["nc"]
    in_maps = _host_prep(**{k: np.asarray(v) for k, v in inputs.items()})
    res = run_bass_kernel_spmd(nc, in_maps, list(range(NCORES)))
    out = np.concatenate(
        [res.results[c]["out"].reshape(BC, S, A) for c in range(NCORES)], axis=0
    )
    return out



# revision 14
# speedup vs baseline: 1.2192x; 1.2192x over previous
"""Trainium2 Bass kernel for nn_ATTN_86543591014439 (dense transformer block).

Reference computation (B=32, S=256, OBS=64, D=1024, H=16 heads, HD=64, A=18):
  h   = x @ W_obs.T + b_obs + pos_emb            [B,S,D]
  qkv = h @ in_w.T + in_b; causal 16-head self-attention
  o   = attn_out @ out_w.T + out_b;  h = h + relu(o)
  f   = relu(h @ w1.T + b1) @ w2.T + b2;  h = h + relu(f)
  out = h @ wa.T + ba                            [B,S,A]

Strategy: data-parallel over batch. 8 cores x 4 sequences (T=1024 token rows
per core), weights replicated, no collectives. All activations stay in SBUF in
feature-major layout ("xT" = [feat, tok]) as bfloat16; matmuls run bf16
(1 PE cycle/row, same rate as fp32r, half the SBUF/DMA/LDWEIGHTS cost),
accumulation stays fp32 in PSUM (end-to-end rel err ~5e-3 vs 2e-2 budget).

Attention per (seq b, head h): the causal mask is applied as an additive
-240 bias PRE-LOADED into the scores PSUM bank by an identity-stationary
matmul (has_written semantics: first_mm clears the bank, later matmuls
accumulate where written / overwrite where not), so exp needs no masking
ops at all (gpsimd freed).  scoresT[k,q] = k-slice.T @ q-slice with the
k>=128 chunk restricted to q>=128 (its q<128 block is dead by causality),
giving a compact [128,384] exp.  Token-major V carries an appended ones
column so the accumulating attnV matmul also yields softmax denominators
(psum row 64); denominators of a head-pair land on partitions {0,64},
one reciprocal_approx_fast (~18 bits, 5x faster than reciprocal) serves
both, a K=1 ones matmul broadcasts the recip rows into a single PSUM bank
(partitions 0:64 / 64:128 via tile_position), one ACT copy moves it to
SBUF, and two DVE muls write the normalized oT slices.

Final head: wa is the STATIONARY operand ([128,18] chunks -> trivial
LDWEIGHTS), output computed as [A, T] and transposed on the host.
"""

import numpy as np
import ml_dtypes

import concourse.tile as tile
from concourse import bacc, mybir
from concourse.bass_utils import run_bass_kernel_spmd

F32 = mybir.dt.float32
F32R = mybir.dt.float32r
BF16 = mybir.dt.bfloat16

B, S, OBS, D, H, A = 32, 256, 64, 1024, 16, 18
HD = D // H
NCORES = 8
BC = B // NCORES  # sequences per core
T = BC * S  # token rows per core (1024)
KC = D // 128  # 128-chunks over D
AF = mybir.ActivationFunctionType

_cache = {}


def _build_nc():
    nc = bacc.Bacc()

    def inp(name, shape, dtype=BF16):
        return nc.declare_dram_parameter(name, list(shape), dtype, isOutput=False).ap()

    xT_e = inp("xT", [OBS, T])
    wobs_e = inp("wobsT", [OBS, D])
    pos_e = inp("posT4b", [D, S], F32)
    wqk_e = inp("wqk_c", [16, 128, KC, 128])
    inbq_e = inp("inb_q", [128, KC], F32)
    wv_e = inp("wv_c", [2, 128, KC, 512])
    wo_e = inp("wo_c", [KC, 128, KC, 128])
    outb_e = inp("outb", [128, KC], F32)
    w1_e = inp("w1_c", [32, 128, KC, 128])
    b1_e = inp("b1", [128, 32], F32)
    w2_e = inp("w2_c", [KC, 128, 32, 128])
    b2_e = inp("b2", [128, KC], F32)
    wa_e = inp("wa_c", [128, KC, A])
    baP_e = inp("baP", [A, 1], F32)
    tm_e = inp("maskTM", [128, 128])
    iden_e = inp("iden", [128, 128])
    ones_v_e = inp("ones_v", [128, H])
    onesb_e = inp("onesb", [128, 64], F32R)
    out_e = nc.declare_dram_parameter("outT", [A, T], F32, isOutput=True).ap()

    with tile.TileContext(nc) as tc:
        with (
            tc.tile_pool(name="cpool", bufs=1) as cpool,
            tc.tile_pool(name="htp", bufs=1) as htp,
            tc.tile_pool(name="rpool", bufs=3) as rpool,
        ):
            # const tiles allocated up front; their DMAs are emitted after
            # phase E so E's xT/wobs/pos transfers lead the DMA queue
            tm = cpool.tile([128, 128], BF16)
            iden = cpool.tile([128, 128], BF16)
            onesB = cpool.tile([128, 64], F32R)
            inbq = cpool.tile([128, KC], F32)
            outb = cpool.tile([128, KC], F32)
            b1 = cpool.tile([128, 32], F32)
            b2 = cpool.tile([128, KC], F32)
            baP = cpool.tile([A, 1], F32)
            wa = cpool.tile([128, KC, A], BF16)

            ht = [
                htp.tile([128, T], BF16, tag=f"ht{m}", name=f"ht{m}")
                for m in range(KC)
            ]

            with (
                tc.tile_pool(name="qkp", bufs=1) as qkp,
                tc.tile_pool(name="vtp", bufs=1) as vtp,
            ):
                qk = [
                    qkp.tile([128, T], BF16, tag=f"qk{m}", name=f"qk{m}")
                    for m in range(16)
                ]
                vt = [
                    vtp.tile([128, H, HD + 1], BF16, tag=f"vt{m}", name=f"vt{m}")
                    for m in range(8)
                ]

                with (
                    tc.tile_pool(name="psg1", bufs=2, space="PSUM") as psg1,
                    tc.tile_pool(name="wvp", bufs=2) as wvp,
                ):
                    wv = [
                        wvp.tile([128, KC, 512], BF16, tag="wv", name=f"wv{vc}")
                        for vc in range(2)
                    ]
                    # ---- E: hT = W_obs @ xT + (pos + b_obs) ----
                    with (
                        nc.named_scope("E"),
                        tc.tile_pool(name="exw", bufs=1) as exw,
                        tc.tile_pool(name="ppos", bufs=8) as ppos,
                    ):
                        xT = exw.tile([OBS, T], BF16)
                        nc.sync.dma_start(out=xT, in_=xT_e)
                        wobs = exw.tile([OBS, D], BF16)
                        nc.sync.dma_start(out=wobs, in_=wobs_e)
                        poss = []
                        for m in range(KC):
                            pos = ppos.tile(
                                [128, S], F32, tag="pos", name=f"pos{m}"
                            )
                            nc.sync.dma_start(
                                out=pos, in_=pos_e[m * 128 : (m + 1) * 128, 0:S]
                            )
                            poss.append(pos)
                        # prefetch V's first weight block under phase E
                        nc.sync.dma_start(out=wv[0], in_=wv_e[0])
                        for m in range(KC):
                            pos = poss[m]
                            for tcol in range(T // 512):
                                sl = slice(tcol * 512, (tcol + 1) * 512)
                                ps = psg1.tile([128, 512], F32, tag="ps")
                                nc.tensor.matmul(
                                    ps,
                                    wobs[:, m * 128 : (m + 1) * 128],
                                    xT[:, sl],
                                    start=True,
                                    stop=True,
                                )
                                for q in range(2):
                                    nc.vector.tensor_add(
                                        ht[m][:, 2 * tcol * S + q * S : 2 * tcol * S + (q + 1) * S],
                                        ps[:, q * S : (q + 1) * S],
                                        pos,
                                    )

                    nc.sync.dma_start(out=inbq, in_=inbq_e)
                    nc.sync.dma_start(out=outb, in_=outb_e)
                    nc.sync.dma_start(out=b1, in_=b1_e)
                    nc.sync.dma_start(out=b2, in_=b2_e)
                    nc.sync.dma_start(out=baP, in_=baP_e)
                    nc.sync.dma_start(out=wa, in_=wa_e)

                    # ---- V: token-major v = h @ Wv.T, with ones column ----
                    with nc.named_scope("V"):
                        for mt in range(8):
                            nc.sync.dma_start(
                                out=vt[mt][:, :, HD : HD + 1],
                                in_=ones_v_e.unsqueeze(2),
                            )
                        nc.sync.dma_start(out=wv[1], in_=wv_e[1])
                        for vc in range(2):
                            for mt in range(8):
                                ps = psg1.tile([128, 512], F32, tag="ps")
                                for k in range(KC):
                                    nc.tensor.matmul(
                                        ps,
                                        ht[k][:, mt * 128 : (mt + 1) * 128],
                                        wv[vc][:, k, :],
                                        start=(k == 0),
                                        stop=(k == KC - 1),
                                    )
                                nc.scalar.activation(
                                    out=vt[mt][:, 8 * vc : 8 * vc + 8, 0:HD],
                                    in_=ps.rearrange("p (h d) -> p h d", d=HD),
                                    func=AF.Copy,
                                )

                with tc.tile_pool(name="otp", bufs=1) as otp:
                    ot = [
                        otp.tile([128, T], BF16, tag=f"ot{m}", name=f"ot{m}")
                        for m in range(KC)
                    ]

                    # ---- QA: Q blocks interleaved with attention pairs.
                    # Pairs for head-pair hh immediately follow Q(hh)/Q(8+hh)
                    # (earlier program order = higher priority), so later Q
                    # blocks act as PE gap-fillers under the attention chains.
                    with (
                        nc.named_scope("QA"),
                        tc.tile_pool(name="wqkp", bufs=3) as wqkp,
                        tc.tile_pool(name="apool", bufs=1) as apool,
                        tc.tile_pool(name="aps", bufs=4, space="PSUM") as aps,
                        tc.tile_pool(name="apu", bufs=2, space="PSUM") as apu,
                        tc.tile_pool(name="apb", bufs=1, space="PSUM") as apb,
                    ):
                        nc.sync.dma_start(out=tm, in_=tm_e)
                        nc.sync.dma_start(out=iden, in_=iden_e)
                        nc.sync.dma_start(out=onesB, in_=onesb_e)

                        def emit_q_block(m, with_bias):
                            wqk = wqkp.tile(
                                [128, KC, 128], BF16, tag="wqk", name=f"wqk{m}"
                            )
                            nc.sync.dma_start(out=wqk, in_=wqk_e[m])
                            for tcol in range(T // 512):
                                sl = slice(tcol * 512, (tcol + 1) * 512)
                                ps = aps.tile(
                                    [128, 512], F32, tag="ps_s", name=f"psq{m}_{tcol}"
                                )
                                for k in range(KC):
                                    nc.tensor.matmul(
                                        ps,
                                        wqk[:, k, :],
                                        ht[k][:, sl],
                                        start=(k == 0),
                                        stop=(k == KC - 1),
                                    )
                                if with_bias:
                                    nc.scalar.activation(
                                        out=qk[m][:, sl],
                                        in_=ps,
                                        func=AF.Identity,
                                        bias=inbq[:, m : m + 1],
                                        scale=1.0,
                                    )
                                else:
                                    # k-bias is softmax-invariant: dropped
                                    nc.scalar.activation(
                                        out=qk[m][:, sl], in_=ps, func=AF.Copy
                                    )

                        for hh in range(8):
                            emit_q_block(hh, True)
                            emit_q_block(8 + hh, False)
                            psu_g = [None] * 2
                            scoll = None
                            qt = qk[hh]
                            kt = qk[8 + hh]
                            for pi in range(2 * BC):
                                b, g = pi // 2, pi % 2
                                idx = pi % 2
                                h = 2 * hh + g
                                r0 = g * 64
                                c0 = b * S
                                ps_s = aps.tile(
                                    [128, 512],
                                    F32,
                                    tag="ps_s",
                                    name=f"ps_s{b}_{h}",
                                )
                                # causal mask as additive -240 PSUM prefill;
                                # first_mm clears the bank, scores accumulate
                                # on the masked regions and overwrite the rest
                                nc.tensor.matmul(
                                    ps_s[:, 0:128], iden, tm,
                                    start=True, stop=False,
                                )
                                nc.tensor.matmul(
                                    ps_s[:, 256:384], iden, tm,
                                    start=False, stop=False,
                                )
                                # scores chunk0: k 0:128 x q 0:256
                                nc.tensor.matmul(
                                    ps_s[:, 0:256],
                                    kt[r0 : r0 + 64, c0 : c0 + 128],
                                    qt[r0 : r0 + 64, c0 : c0 + 256],
                                    start=False,
                                    stop=False,
                                )
                                # scores chunk1: k 128:256 x q 128:256 only
                                # (q<128 is fully masked -> never computed)
                                nc.tensor.matmul(
                                    ps_s[:, 256:384],
                                    kt[r0 : r0 + 64, c0 + 128 : c0 + 256],
                                    qt[r0 : r0 + 64, c0 + 128 : c0 + 256],
                                    start=False,
                                    stop=True,
                                )
                                expT = apool.tile(
                                    [128, 384],
                                    BF16,
                                    tag="expT", bufs=5,
                                    name=f"expT{b}_{h}",
                                )
                                nc.scalar.activation(
                                    out=expT,
                                    in_=ps_s[:, 0:384],
                                    func=AF.Exp,
                                    scale=0.125,
                                )
                                ps_u = apu.tile(
                                    [65, 256], F32, tag="ps_ub", name=f"ps_u{b}_{h}"
                                )
                                nc.tensor.matmul(
                                    ps_u,
                                    vt[2 * b][:, h, :],
                                    expT[:, 0:256],
                                    start=True,
                                    stop=False,
                                )
                                nc.tensor.matmul(
                                    ps_u[:, 128:256],
                                    vt[2 * b + 1][:, h, :],
                                    expT[:, 256:384],
                                    start=False,
                                    stop=True,
                                )
                                psu_g[idx] = ps_u
                                # gather the two sums rows (psum partition 64)
                                # onto scoll partitions {0,64}: one reciprocal
                                # serves both heads of the pair
                                if idx == 0:
                                    scoll = apool.tile(
                                        [33, 256],
                                        F32R,
                                        tag="scoll", bufs=2,
                                        name=f"scoll{b}_{hh}",
                                    )
                                    nc.scalar.activation(
                                        out=scoll[0:1, :],
                                        in_=ps_u[64:65, :],
                                        func=AF.Copy,
                                    )
                                    continue
                                nc.vector.tensor_copy(
                                    out=scoll[32:33, :], in_=ps_u[64:65, :]
                                )
                                rcoll = apool.tile(
                                    [33, 256], F32R, tag="rcoll", bufs=2,
                                    name=f"rc{b}_{hh}",
                                )
                                # reciprocal_approx_fast (~18 bits, ~5x
                                # faster than reciprocal) but with an F32R
                                # output so the DVE writeback rounds — the
                                # verifier requires fp32r matmul operands to
                                # be produced rounded
                                from concourse.dve_ops import (
                                    RECIP_APPROX_FAST_CONSTS,
                                    RECIPROCAL_APPROX_FAST,
                                )

                                _c = RECIP_APPROX_FAST_CONSTS
                                nc.vector._custom_dve(
                                    RECIPROCAL_APPROX_FAST,
                                    out=rcoll,
                                    in0=scoll.bitcast(F32),
                                    s0=_c["s0"],
                                    s1=_c["s1"],
                                    imm2=_c["imm2"],
                                )
                                for gj in range(2):
                                    rj = gj * 32
                                    ps_b = apb.tile(
                                        [64, 256], F32, tag="ps_b", bufs=2,
                                        name=f"ps_b{b}_{hh}_{gj}",
                                    )
                                    nc.tensor.matmul(
                                        ps_b,
                                        onesB[rj : rj + 1, 0:64],
                                        rcoll[rj : rj + 1, :],
                                        start=True,
                                        stop=True,
                                        tile_position=(rj, 0),
                                    )
                                    recipB = apool.tile(
                                        [64, 256], F32, tag="recipB", bufs=3,
                                        name=f"rB{b}_{hh}_{gj}",
                                    )
                                    if gj == 0:
                                        nc.scalar.activation(
                                            out=recipB, in_=ps_b, func=AF.Copy
                                        )
                                    else:
                                        nc.vector.tensor_copy(
                                            out=recipB, in_=ps_b
                                        )
                                    nc.vector.tensor_mul(
                                        ot[hh][gj * 64 : gj * 64 + 64, b * S : (b + 1) * S],
                                        psu_g[gj][0:64, :],
                                        recipB,
                                    )

                    with (
                        nc.named_scope("P"),
                        tc.tile_pool(name="wop", bufs=3) as wop,
                        tc.tile_pool(name="psp", bufs=4, space="PSUM") as psp,
                    ):
                        for m in range(KC):
                            wo = wop.tile([128, KC, 128], BF16, tag="wo")
                            nc.sync.dma_start(out=wo, in_=wo_e[m])
                            for tcol in range(T // 512):
                                sl = slice(tcol * 512, (tcol + 1) * 512)
                                ps = psp.tile([128, 512], F32, tag="ps")
                                for k in range(KC):
                                    nc.tensor.matmul(
                                        ps,
                                        wo[:, k, :],
                                        ot[k][:, sl],
                                        start=(k == 0),
                                        stop=(k == KC - 1),
                                    )
                                rt = rpool.tile([128, 512], BF16, tag="rt")
                                nc.scalar.activation(
                                    out=rt,
                                    in_=ps,
                                    func=AF.Relu,
                                    bias=outb[:, m : m + 1],
                                    scale=1.0,
                                )
                                nc.vector.tensor_add(
                                    ht[m][:, sl], ht[m][:, sl], rt
                                )

            # ---- F1/F2: FFN (otp/qkp/vtp scopes closed, SBUF freed) ----
            with tc.tile_pool(name="gtp", bufs=1) as gtp:
                gt = [
                    gtp.tile([128, T], BF16, tag=f"gt{m}", name=f"gt{m}")
                    for m in range(32)
                ]
                with tc.tile_pool(name="psg3", bufs=4, space="PSUM") as psg3:
                    with (
                        nc.named_scope("F1"),
                        tc.tile_pool(name="w1p", bufs=3) as w1p,
                    ):
                        for m in range(32):
                            w1m = w1p.tile([128, KC, 128], BF16, tag="w1m")
                            nc.sync.dma_start(out=w1m, in_=w1_e[m])
                            for tcol in range(T // 512):
                                sl = slice(tcol * 512, (tcol + 1) * 512)
                                ps = psg3.tile([128, 512], F32, tag="ps")
                                for k in range(KC):
                                    nc.tensor.matmul(
                                        ps,
                                        w1m[:, k, :],
                                        ht[k][:, sl],
                                        start=(k == 0),
                                        stop=(k == KC - 1),
                                    )
                                nc.scalar.activation(
                                    out=gt[m][:, sl],
                                    in_=ps,
                                    func=AF.Relu,
                                    bias=b1[:, m : m + 1],
                                    scale=1.0,
                                )

                    with (
                        nc.named_scope("F2"),
                        tc.tile_pool(name="w2p", bufs=2) as w2p,
                    ):
                        for m in range(KC):
                            w2m = w2p.tile([128, 32, 128], BF16, tag="w2m")
                            nc.sync.dma_start(out=w2m, in_=w2_e[m])
                            for tcol in range(T // 512):
                                sl = slice(tcol * 512, (tcol + 1) * 512)
                                ps = psg3.tile([128, 512], F32, tag="ps")
                                for k in range(32):
                                    nc.tensor.matmul(
                                        ps,
                                        w2m[:, k, :],
                                        gt[k][:, sl],
                                        start=(k == 0),
                                        stop=(k == 31),
                                    )
                                rt = rpool.tile([128, 512], BF16, tag="rt")
                                nc.scalar.activation(
                                    out=rt,
                                    in_=ps,
                                    func=AF.Relu,
                                    bias=b2[:, m : m + 1],
                                    scale=1.0,
                                )
                                nc.vector.tensor_add(ht[m][:, sl], ht[m][:, sl], rt)

                    # ---- H: outT = wa.T-stationary head, [A, T] layout ----
                    with (
                        nc.named_scope("H"),
                        tc.tile_pool(name="houtp", bufs=1) as houtp,
                    ):
                        outA = houtp.tile([A, T], F32)
                        for tcol in range(T // 512):
                            sl = slice(tcol * 512, (tcol + 1) * 512)
                            ps = psg3.tile([A, 512], F32, tag="psh", bufs=2)
                            for k in range(KC):
                                nc.tensor.matmul(
                                    ps,
                                    wa[:, k, :],
                                    ht[k][:, sl],
                                    start=(k == 0),
                                    stop=(k == KC - 1),
                                )
                            nc.scalar.activation(
                                out=outA[:, sl],
                                in_=ps,
                                func=AF.Identity,
                                bias=baP,
                                scale=1.0,
                            )
                        nc.sync.dma_start(out=out_e, in_=outA)

    nc.compile()
    return nc


def _host_prep(
    x, pos_emb, W_obs, b_obs, in_w, in_b, out_w, out_b, w1, b1, w2, b2, wa, ba
):
    f = np.float32
    bf = ml_dtypes.bfloat16

    def cbf(a):
        return np.ascontiguousarray(np.asarray(np.asarray(a, f), bf))

    wqkT = np.asarray(in_w[: 2 * D], f).T  # [D, 2D]
    wvT = np.asarray(in_w[2 * D :], f).T  # [D, D]
    woT = np.asarray(out_w, f).T
    w1T = np.asarray(w1, f).T  # [D, 4D]
    w2T = np.asarray(w2, f).T  # [4D, D]
    waT = np.asarray(wa, f).T  # [D, A]

    kidx = np.arange(128)[:, None]
    qidx = np.arange(128)[None, :]
    tm = np.where(kidx <= qidx, f(0.0), f(-240.0))

    shared = {
        "wobsT": cbf(W_obs.T),
        "wqk_c": cbf(wqkT.reshape(KC, 128, 16, 128).transpose(2, 1, 0, 3)),
        "inb_q": np.ascontiguousarray(
            np.asarray(in_b[:D], f).reshape(KC, 128).T
        ),
        "wv_c": cbf(wvT.reshape(KC, 128, 2, 512).transpose(2, 1, 0, 3)),
        "wo_c": cbf(woT.reshape(KC, 128, KC, 128).transpose(2, 1, 0, 3)),
        "w1_c": cbf(w1T.reshape(KC, 128, 32, 128).transpose(2, 1, 0, 3)),
        "b1": np.ascontiguousarray(np.asarray(b1, f).reshape(32, 128).T),
        "w2_c": cbf(w2T.reshape(32, 128, KC, 128).transpose(2, 1, 0, 3)),
        "b2": np.ascontiguousarray(np.asarray(b2, f).reshape(KC, 128).T),
        "wa_c": cbf(waT.reshape(KC, 128, A).transpose(1, 0, 2)),
        "baP": np.ascontiguousarray(np.asarray(ba, f).reshape(A, 1)),
        "maskTM": cbf(tm),
        "iden": cbf(np.eye(128, dtype=f)),
        "ones_v": np.ones((128, H), bf),
        "onesb": np.ones((128, 64), f),
    }
    # v-bias commutes through attention (rows of attn sum to 1):
    # out_b_eff = out_b + bv @ out_w.T
    out_b_eff = out_b + in_b[2 * D :] @ out_w.T
    shared["outb"] = np.ascontiguousarray(
        np.asarray(out_b_eff, f).reshape(KC, 128).T, dtype=f
    )
    posT = np.asarray(pos_emb[0].T, f) + np.asarray(b_obs, f)[:, None]
    shared["posT4b"] = np.ascontiguousarray(posT, dtype=f)

    in_maps = []
    for c in range(NCORES):
        m = dict(shared)
        xc = np.asarray(x[c * BC : (c + 1) * BC], f).reshape(T, OBS)
        m["xT"] = cbf(xc.T)
        in_maps.append(m)
    return in_maps


def kernel(**inputs):
    if "nc" not in _cache:
        _cache["nc"] = _build_nc()
    nc = _cache["nc"]
    in_maps = _host_prep(**{k: np.asarray(v) for k, v in inputs.items()})
    res = run_bass_kernel_spmd(nc, in_maps, list(range(NCORES)))
    out = np.concatenate(
        [
            np.asarray(res.results[c]["outT"], np.float32).T.reshape(BC, S, A)
            for c in range(NCORES)
        ],
        axis=0,
    )
    return out
